# revision 2
# baseline (speedup 1.0000x reference)
"""Trainium2 Bass kernel for the 50-step autoregressive MLP rollout.

Per batch row b: state = x[b,0,2:9]; for t: h1 = tanh([u_t, s] @ W1);
h2 = tanh(h1 @ W2); s += DT * (h2 @ W3); out[b,t] = s.

Strategy (pure data parallel, 4096 rows/core on 8 cores):
- Feature-major on chip.  Batch in 8 chunks of 512; chunk c = (h, q) with
  half h = c//4 and quadrant q = c%4.
- The integrator state lives in PSUM for the whole scan as z = state/DT
  ([103, 512] f32 per half: quadrant q's 7 state rows at partitions 32q..32q+6).
  Layer 3 (h2 @ W3, W3 at natural fp8 scale) accumulates straight into z with
  start=False; the 1/DT scaling makes fp8 W3 storage accurate (DT*W3 would sit
  in e4m3's denormal range and quantize catastrophically).
- Each step one DVE tensor_scalar per half copies z*DT -> bf16 sbuf tile sT,
  which doubles as layer-1 moving operand and as the DMA'd output.
- Layer 1 runs in bf16 (state rows K=7 + control rows K=2 accumulated, per
  chunk quadrant via tile_position).  Layer 2 runs fp8 DoubleRow (two K-tiles
  per instruction at 0.5 cyc/row).  Layer 3 fp8 DoubleRow or bf16 (config).
- Controls for all steps are host-transposed and DMA'd per step into a
  [128, 1024] bf16 tile (rows 32q..32q+1, halves side by side), issued from
  the GpSimd queue (25ns vs 565ns on SP).
- Host does all packing/unpacking (transposes, dtype casts, output reshape);
  only the scan itself runs on-device.  Biases are zeros per the spec and
  asserted so.
"""

import numpy as np

B_TOTAL = 32768
N_CORES = 8
B_CORE = B_TOTAL // N_CORES      # 4096
H = 50
F = 9
NCTRL = 2
NST = 7
HID = 256
DT = 0.02
NT = 512                         # chunk batch size
NCH = B_CORE // NT               # 8 chunks

_CACHE = {}


def _build(horizon=H, pade_chunks=0):
    import concourse.bacc as bacc
    import concourse.mybir as mybir
    import concourse.tile as tile

    f32 = mybir.dt.float32
    f32r = mybir.dt.float32r
    bf16 = mybir.dt.bfloat16
    f8 = mybir.dt.float8e4
    Tanh = mybir.ActivationFunctionType.Tanh
    DR = mybir.MatmulPerfMode.DoubleRow
    mult = mybir.AluOpType.mult

    nc = bacc.Bacc("TRN2", target_bir_lowering=False, debug=False,
                   num_devices=N_CORES)

    w1s_d = nc.dram_tensor("w1s", [128, 256], bf16, kind="ExternalInput").ap()
    w1u_d = nc.dram_tensor("w1u", [128, 512], f8, kind="ExternalInput").ap()
    w2_d = nc.dram_tensor("w2", [128, 1024], f8, kind="ExternalInput").ap()
    w3_d = nc.dram_tensor("w3", [128, 14], bf16, kind="ExternalInput").ap()
    i103_d = nc.dram_tensor("i103", [103, 104], f32r,
                            kind="ExternalInput").ap()
    s0_d = nc.dram_tensor("s0", [256, 512], f32r, kind="ExternalInput").ap()
    ctrl_d = nc.dram_tensor("ctrl", [horizon * 8, 1024], f8,
                            kind="ExternalInput").ap()
    out_d = nc.dram_tensor("out", [horizon * 2 * 103, 512], bf16,
                           kind="ExternalOutput").ap()

    with tile.TileContext(nc) as tc:
        with (
            tc.tile_pool(name="persist", bufs=1) as pp,
            tc.tile_pool(name="sT", bufs=6) as sp_,
            tc.tile_pool(name="uT", bufs=3) as up_,
            tc.tile_pool(name="h1", bufs=4) as h1p,
            tc.tile_pool(name="h2", bufs=3) as h2p,
            tc.tile_pool(name="psA", bufs=3, space="PSUM") as psA,
            tc.tile_pool(name="psZ", bufs=1, space="PSUM") as psZ,
        ):
            w1s_sb = pp.tile([128, 256], bf16, tag="w1s")
            w1u_sb = pp.tile([128, 512], f8, tag="w1u")
            w2sb = pp.tile([128, 1024], f8, tag="w2")
            w3sb = pp.tile([128, 14], bf16, tag="w3")
            i1sb = pp.tile([103, 104], f32r, tag="i103")
            s0sb = pp.tile([128, 1024], f32r, tag="s0")

            nc.sync.dma_start(w1s_sb[:, :], w1s_d)
            nc.sync.dma_start(w1u_sb[:, :], w1u_d)
            nc.sync.dma_start(w2sb[:, :], w2_d)
            nc.sync.dma_start(w3sb[:, :], w3_d)
            nc.sync.dma_start(i1sb[:, :], i103_d)
            nc.sync.dma_start(s0sb[:, 0:512], s0_d[0:128, :])
            nc.sync.dma_start(s0sb[:, 512:1024], s0_d[128:256, :])

            z = [psZ.tile([103, 512], f32, tag=f"z{h}", name=f"z{h}")
                 for h in (0, 1)]
            for h in (0, 1):
                # z0 = (I/DT).T @ state0  (i103 holds eye(103)/DT)
                nc.tensor.matmul(z[h][0:103, :], i1sb[0:103, 0:103],
                                 s0sb[0:103, h * 512:(h + 1) * 512],
                                 start=True, stop=True)

            def new_uT(t):
                ut = up_.tile([128, 1024], f8, tag="uT", name=f"uT{t}")
                for q in range(4):
                    nc.sync.dma_start(
                        ut[32 * q:32 * q + 2, :],
                        ctrl_d[(t * 4 + q) * 2:(t * 4 + q) * 2 + 2, :])
                return ut

            def copy_state(t, h):
                # sT(t)[h] = z[h] * DT in bf16; doubles as the output row
                st = sp_.tile([128, 512], bf16, tag="sT", name=f"sT{t}_{h}")
                nc.vector.tensor_scalar(st[0:103, :], z[h][0:103, :],
                                        DT, None, op0=mult)
                return st

            cur_sT = {h: copy_state(0, h) for h in (0, 1)}
            cur_uT = new_uT(0)
            nxt_uT = new_uT(1) if horizon > 1 else None
            hold = {}

            def stage1(t, c):
                h, q = c // 4, c % 4
                r = 32 * q
                ph1 = psA.tile([128, 1024], f32, tag="ph1", name="ph1")
                uv = cur_uT[r:r + NCTRL, h * 512:(h + 1) * 512].unsqueeze(
                    1).broadcast_to([NCTRL, 2, 512])
                for m in (0, 1):
                    nc.tensor.matmul(
                        ph1[:, m * 512:(m + 1) * 512],
                        w1u_sb[r:r + NCTRL, m * 256:(m + 1) * 256].rearrange(
                            "k (i m2) -> k i m2", i=2),
                        uv, start=True, stop=False, perf_mode=DR,
                        tile_position=(r, 0))
                    nc.tensor.matmul(
                        ph1[:, m * 512:(m + 1) * 512],
                        w1s_sb[r:r + NST, m * 128:(m + 1) * 128],
                        cur_sT[h][r:r + NST, :],
                        start=False, stop=True, tile_position=(r, 0))
                h1t = h1p.tile([128, 1024], f8, tag="h1")
                nc.scalar.activation(h1t[:, :], ph1[:, :], Tanh)
                hold[c] = h1t

            def stage2(t, c):
                h, q = c // 4, c % 4
                r = 32 * q
                h1t = hold.pop(c)
                ph2 = psA.tile([128, 1024], f32, tag="ph1", name="ph2")
                h1v = h1t[:, :].rearrange("k (i n) -> k i n", i=2)
                for m in (0, 1):
                    for rphase in (0, 1):  # fp8 weight + fp8 residual
                        nc.tensor.matmul(
                            ph2[:, m * 512:(m + 1) * 512],
                            w2sb[:, m * 512 + rphase * 256:
                                 m * 512 + (rphase + 1) * 256].rearrange(
                                "k (i m2) -> k i m2", i=2),
                            h1v, start=(rphase == 0), stop=(rphase == 1),
                            perf_mode=DR)
                h2t = h2p.tile([128, 1024], bf16, tag="h2")
                nc.scalar.activation(h2t[:, :], ph2[:, :], Tanh)
                for i in (0, 1):
                    nc.tensor.matmul(
                        z[h][r:r + NST, :],
                        w3sb[:, i * NST:(i + 1) * NST],
                        h2t[:, i * 512:(i + 1) * 512],
                        start=False, stop=(i == 1),
                        skip_group_check=True, tile_position=(0, r))

            LAG = 2  # stage2 trails stage1 by LAG chunks for ACT slack
            for t in range(horizon):
                nxt_sT = {}
                for c in range(NCH):
                    if c >= LAG:
                        stage2(t, c - LAG)
                    stage1(t, c)
                    if c == 3 + LAG:
                        nxt_sT[0] = copy_state(t + 1, 0)
                        nc.sync.dma_start(
                            out_d[(t * 2) * 103:(t * 2) * 103 + 103, :],
                            nxt_sT[0][0:103, :])
                for c in range(NCH - LAG, NCH):
                    stage2(t, c)
                nxt_sT[1] = copy_state(t + 1, 1)
                nc.sync.dma_start(
                    out_d[(t * 2 + 1) * 103:(t * 2 + 1) * 103 + 103, :],
                    nxt_sT[1][0:103, :])
                cur_sT = nxt_sT
                cur_uT = nxt_uT
                if t + 2 < horizon:
                    nxt_uT = new_uT(t + 2)

    nc.compile()
    return nc


def _get_nc(**kw):
    key = tuple(sorted(kw.items()))
    if key not in _CACHE:
        _CACHE[key] = _build(**kw)
    return _CACHE[key]


def _pack_inputs(x, W1, W2, W3):
    import ml_dtypes
    f8 = ml_dtypes.float8_e4m3
    bf16 = ml_dtypes.bfloat16

    w1s = np.zeros((128, 256), np.float32)
    w1u = np.zeros((128, 512), np.float32)
    for q in range(4):
        w1s[32 * q:32 * q + NST, :] = W1[NCTRL:F, :]
        for m in (0, 1):
            for i in (0, 1):
                w1u[32 * q:32 * q + NCTRL,
                    m * 256 + i * 128:m * 256 + (i + 1) * 128] = \
                    0.5 * W1[0:NCTRL, m * 128:(m + 1) * 128]
    def pack_res(Wfull, blk_m):
        # [128, 2*2*blk_m] fp8 A|R per m-half: (m, phase, ktile, blk_m)
        nm = Wfull.shape[1] // blk_m
        A = Wfull.astype(f8).astype(np.float32)
        R = (Wfull - A)
        outw = np.zeros((128, nm * 2 * 2 * blk_m), np.float32)
        for m in range(nm):
            for phase, Wp in ((0, A), (1, R)):
                for i in (0, 1):
                    c0 = m * 2 * 2 * blk_m + phase * 2 * blk_m + i * blk_m
                    outw[:, c0:c0 + blk_m] = \
                        Wp[i * 128:(i + 1) * 128, m * blk_m:(m + 1) * blk_m]
        return outw
    w2 = pack_res(W2, 128)
    w3 = np.zeros((128, 14), np.float32)
    for i in (0, 1):
        w3[:, i * NST:(i + 1) * NST] = W3[i * 128:(i + 1) * 128, :]
    i103 = np.zeros((103, 104), np.float32)
    i103[:, 0:103] = np.eye(103, dtype=np.float32) / DT

    # per-core tensors
    xs = x.reshape(N_CORES, B_CORE, H, F)
    s0 = np.zeros((N_CORES, 256, 512), np.float32)
    ctrl = np.zeros((N_CORES, H * 8, 1024), np.float32)
    for c in range(NCH):
        h, q = c // 4, c % 4
        blk = xs[:, c * NT:(c + 1) * NT]          # [8, 512, H, F]
        s0[:, h * 128 + 32 * q:h * 128 + 32 * q + NST, :] = \
            blk[:, :, 0, NCTRL:F].transpose(0, 2, 1)
        # ctrl rows (t, q, j), cols h*512+n
        ctrl[:, :, h * 512:(h + 1) * 512].reshape(
            N_CORES, H, 4, 2, 512)[:, :, q, :, :] = \
            blk[:, :, :, 0:NCTRL].transpose(0, 2, 3, 1)
    return {
        "w1s": w1s.astype(bf16),
        "w1u": w1u.astype(f8),
        "w2": w2.astype(f8),
        "w3": w3.astype(bf16),
        "i103": i103,
        "s0": s0,
        "ctrl": ctrl.astype(f8),
    }


def _unpack_output(res):
    out = np.empty((B_TOTAL, H, NST), np.float32)
    outs = out.reshape(N_CORES, NCH, NT, H, NST)
    for core in range(N_CORES):
        o = np.asarray(res.results[core]["out"]).astype(np.float32)
        o = o.reshape(H, 2, 103, 512)
        for c in range(NCH):
            h, q = c // 4, c % 4
            outs[core, c] = o[:, h, 32 * q:32 * q + NST, :].transpose(2, 0, 1)
    return out


def _run(x, W1, b1, W2, b2, W3, b3, **spmd_kwargs):
    import concourse.bass_utils as bass_utils

    x = np.ascontiguousarray(np.asarray(x, dtype=np.float32))
    W1 = np.asarray(W1, dtype=np.float32)
    W2 = np.asarray(W2, dtype=np.float32)
    W3 = np.asarray(W3, dtype=np.float32)
    for b in (b1, b2, b3):
        assert not np.any(np.asarray(b)), "kernel built for zero biases"

    nc = _get_nc()
    packed = _pack_inputs(x, W1, W2, W3)
    shared = {k: packed[k] for k in ("w1s", "w1u", "w2", "w3", "i103")}
    in_maps = []
    for c in range(N_CORES):
        m = dict(shared)
        m["s0"] = packed["s0"][c]
        m["ctrl"] = packed["ctrl"][c]
        in_maps.append(m)
    res = bass_utils.run_bass_kernel_spmd(nc, in_maps,
                                          core_ids=list(range(N_CORES)),
                                          **spmd_kwargs)
    return _unpack_output(res), res


def kernel(x, W1, b1, W2, b2, W3, b3):
    out, _ = _run(x, W1, b1, W2, b2, W3, b3)
    return out
